# revision 6
# baseline (speedup 1.0000x reference)
"""Trainium2 Bass kernel for DecoupledSOLOHead mask decoding + Matrix NMS.

Math (reference):
    mask_x = seg_preds_x[x_inds]; mask_y = seg_preds_y[y_inds]   # [N,H,W]
    soft = mask_x*mask_y; hard = soft > THR
    sum_masks = hard.sum((1,2)); seg_score = (soft*hard).sum((1,2))/max(sm,1)
    scores = cate_scores * seg_score
    inter = hard_flat @ hard_flat.T          # [N,N]
    ... matrix NMS (gaussian) -> scores * decay_coef

Strategy (8 cores):
  - Shard the H*W=60800 pixel dim: 7600 px/core, zero-padded to 7680 = 60
    chunks of 128 pixels.
  - Per chunk, gather the candidate masks in PIXEL-MAJOR layout [128px, 500]
    directly on the TensorEngine: gx = slab_chunk.T @ onehot_x, where
    slab_chunk is [128 G, 128 px] (G on partitions) and onehot_x[g,i] =
    (x_inds[i]==g).  fp32 matmul is 4x slower than bf16, so the fp32 slab is
    pre-split on host into bf16 hi+lo parts; two bf16 matmuls accumulate
    hi+lo in PSUM.  hi+lo reproduces x to ~2^-18 relative, so the threshold
    compare and soft values match fp32 to ~1e-5 aggregate effect.
  - DVE: soft = gx*gy (fp32), hard = (soft>THR) in bf16 (exact 0/1),
    shsoft = (soft>THR)*soft in bf16.
  - inter partial: 4 accumulated bf16 matmuls per chunk
    s_m += hard[:,125m:125(m+1)].T @ hard  (binary bf16 inputs, fp32 PSUM
    accumulation => exact integer inter).  num += ones.T @ shsoft.
  - sum_masks = diag(inter) extracted via affine_select.
  - One 1MB AllReduce combines [inter | num | sm] across the 8 cores.
  - Decay stage (replicated on every core): with S symmetric, the
    "transposed" orientation S^T[j,i] needed for the axis-0 reductions is
    just S itself, so no transposes are needed.  comp_iou / decay_coef are
    free-dim reductions.  1/comp_matrix = exp(+2*comp^2).  Row<->column
    reorientations of [500] vectors go through tiny DRAM bounces.
"""

import sys

if "/opt/trn_rl_repo" not in sys.path:
    sys.path.insert(0, "/opt/trn_rl_repo")

from contextlib import ExitStack

import numpy as np
import ml_dtypes

import concourse.bass as bass
import concourse.tile as tile
from concourse import bacc, mybir
from concourse.bass_utils import run_bass_kernel_spmd

N = 500
G = 128
H, W = 200, 304
HW = H * W              # 60800
NCORES = 8
PPC = HW // NCORES      # 7600 pixels per core
PAD = 7680              # padded to 60 chunks of 128
CHUNKS = PAD // 128     # 60
MT = 125                # candidate tile (4 tiles of 125 = 500)
THR = 0.005
SIGMA = 2.0

BF16 = mybir.dt.bfloat16
F32 = mybir.dt.float32
ALU = mybir.AluOpType
import bass_rust

AFT = bass_rust.ActivationFunctionType

# cc buffer layout (flat f32):  [S (500*500) | num (500) | sm (500)]
CC_S = 0
CC_NUM = N * N          # 250000
CC_SM = N * N + N       # 250500
CC_LEN = N * N + 2 * N  # 251000

_NC_CACHE = []


def _r2(ap, f):
    """reshape a flat (1-D) AP slice to [p, f]"""
    return ap.rearrange("(p f) -> p f", f=f)


def _build_nc():
    nc = bacc.Bacc("TRN2", target_bir_lowering=False, debug=False,
                   num_devices=NCORES)

    xhi_d = nc.dram_tensor("xhi", [G, PAD], BF16, kind="ExternalInput")
    xlo_d = nc.dram_tensor("xlo", [G, PAD], BF16, kind="ExternalInput")
    yhi_d = nc.dram_tensor("yhi", [G, PAD], BF16, kind="ExternalInput")
    ylo_d = nc.dram_tensor("ylo", [G, PAD], BF16, kind="ExternalInput")
    ohx_d = nc.dram_tensor("ohx", [G, N], BF16, kind="ExternalInput")
    ohy_d = nc.dram_tensor("ohy", [G, N], BF16, kind="ExternalInput")
    # maskt[t][j_local, i] = (labels[i]==labels[125t+j_local]) & (i < 125t+j_local)
    maskt_d = nc.dram_tensor("maskt", [4, MT, N], F32, kind="ExternalInput")
    cate_d = nc.dram_tensor("cate", [1, N], F32, kind="ExternalInput")
    out_d = nc.dram_tensor("out", [1, N], F32, kind="ExternalOutput")

    with tile.TileContext(nc) as tc, ExitStack() as ctx:
        consts = ctx.enter_context(tc.tile_pool(name="consts", bufs=1))
        work = ctx.enter_context(tc.tile_pool(name="work", bufs=3))
        fin = ctx.enter_context(tc.tile_pool(name="fin", bufs=1))
        psS = ctx.enter_context(tc.tile_pool(name="psS", bufs=1, space="PSUM"))
        psG = ctx.enter_context(tc.tile_pool(name="psG", bufs=1, space="PSUM"))
        dram = ctx.enter_context(tc.tile_pool(name="dram", bufs=1, space="DRAM"))

        # ---- load constants / slabs (split slab DMAs for pipelining) ----
        xhi_s = consts.tile([G, PAD], BF16)
        xlo_s = consts.tile([G, PAD], BF16)
        yhi_s = consts.tile([G, PAD], BF16)
        ylo_s = consts.tile([G, PAD], BF16)
        for t, d in ((xhi_s, xhi_d), (yhi_s, yhi_d), (xlo_s, xlo_d),
                     (ylo_s, ylo_d)):
            for p in range(4):
                sl = np.s_[:, p * (PAD // 4):(p + 1) * (PAD // 4)]
                nc.sync.dma_start(t[sl], d[sl])
        ohx_s = consts.tile([G, N], BF16)
        nc.sync.dma_start(ohx_s[:], ohx_d[:])
        ohy_s = consts.tile([G, N], BF16)
        nc.sync.dma_start(ohy_s[:], ohy_d[:])
        maskt_s = []
        for t in range(4):
            mt_ = consts.tile([MT, N], F32, name=f"maskt{t}")
            nc.sync.dma_start(mt_[:], maskt_d[t])
            maskt_s.append(mt_)
        cate_s = consts.tile([1, N], F32)
        nc.sync.dma_start(cate_s[:], cate_d[:])
        ones_s = consts.tile([G, 1], BF16)
        nc.vector.memset(ones_s[:], 1.0)

        # ---- PSUM accumulators: 4 S tiles + num = 5 banks; gx(2)+gy(1) ----
        s_ps = [psS.tile([MT, N], F32, name=f"s_ps{m}") for m in range(4)]
        num_ps = psS.tile([1, N], F32)

        # ---- chunk loop ----
        for c in range(CHUNKS):
            cs = np.s_[:, c * 128:(c + 1) * 128]
            first, last = (c == 0), (c == CHUNKS - 1)
            gx = psG.tile([128, N], F32, tag="gx", bufs=2, name="gx")
            gy = psG.tile([128, N], F32, tag="gy", bufs=1, name="gy")
            nc.tensor.matmul(gx[:], xhi_s[cs], ohx_s[:], start=True, stop=False)
            nc.tensor.matmul(gx[:], xlo_s[cs], ohx_s[:], start=False, stop=True)
            nc.tensor.matmul(gy[:], yhi_s[cs], ohy_s[:], start=True, stop=False)
            nc.tensor.matmul(gy[:], ylo_s[cs], ohy_s[:], start=False, stop=True)

            # DVE cannot read two PSUM operands in one op (walrus verifier
            # rejects it); bounce gx through SBUF on the idle scalar engine.
            gxs = work.tile([128, N], F32, tag="gxs", name="gxs")
            nc.scalar.copy(gxs[:], gx[:])
            soft = work.tile([128, N], F32, tag="soft", name="soft")
            nc.vector.tensor_tensor(soft[:], gxs[:], gy[:], op=ALU.mult)
            hard = work.tile([128, N], BF16, tag="hard", name="hard")
            nc.vector.tensor_scalar(hard[:], soft[:], THR, None, op0=ALU.is_gt)
            shs = work.tile([128, N], BF16, tag="shs", name="shs")
            nc.vector.scalar_tensor_tensor(shs[:], soft[:], THR, soft[:],
                                           op0=ALU.is_gt, op1=ALU.mult)

            for m in range(4):
                nc.tensor.matmul(s_ps[m][:], hard[:, MT * m:MT * (m + 1)],
                                 hard[:], start=first, stop=last)
            nc.tensor.matmul(num_ps[:], ones_s[:], shs[:], start=first,
                             stop=last)

        # ---- epilogue: S/num -> SBUF, sm = diag(S) ----
        ssb = []
        for m in range(4):
            s = fin.tile([MT, N], F32, name=f"ssb{m}")
            nc.vector.tensor_copy(s[:], s_ps[m][:])
            ssb.append(s)
        numrow_l = fin.tile([1, N], F32)
        nc.vector.tensor_copy(numrow_l[:], num_ps[:])
        smcol_l = fin.tile([MT, 4], F32)
        for m in range(4):
            dsel = work.tile([MT, N], F32, tag="dsel", name="dsel")
            # keep in_ where (base + p - f == 0) i.e. f == 125*m + p
            nc.gpsimd.affine_select(out=dsel[:], in_=ssb[m][:],
                                    pattern=[[-1, N]], compare_op=ALU.is_equal,
                                    fill=0.0, base=MT * m, channel_multiplier=1)
            nc.vector.tensor_reduce(smcol_l[:, m:m + 1], dsel[:],
                                    axis=mybir.AxisListType.X, op=ALU.add)

        # ---- AllReduce of [S | num | sm] ----
        cc_in = dram.tile([CC_LEN], F32)
        cc_out = dram.tile([CC_LEN], F32, addr_space="Shared")
        for m in range(4):
            nc.sync.dma_start(
                _r2(cc_in[MT * m * N:(MT * m + MT) * N], N), ssb[m][:])
        nc.sync.dma_start(_r2(cc_in[CC_NUM:CC_NUM + N], N), numrow_l[:])
        for m in range(4):
            nc.sync.dma_start(
                _r2(cc_in[CC_SM + MT * m:CC_SM + MT * (m + 1)], 1),
                smcol_l[:, m:m + 1])
        nc.gpsimd.collective_compute(
            "AllReduce", ALU.add, replica_groups=[list(range(NCORES))],
            ins=[cc_in.opt()], outs=[cc_out.opt()])

        # ---- decay stage (replicated; S symmetric => S^T tiles == S tiles) --
        st = []
        for t in range(4):
            s = fin.tile([MT, N], F32, name=f"st{t}")
            nc.sync.dma_start(s[:], _r2(cc_out[MT * t * N:(MT * t + MT) * N], N))
            st.append(s)
        smb = fin.tile([MT, N], F32)   # sm[i] broadcast down partitions
        nc.gpsimd.dma_start(smb[:], bass.AP(tensor=cc_out.tensor,
                                            offset=cc_out.offset + CC_SM,
                                            ap=[[0, MT], [1, N]]))
        smc, numr, smr = [], None, None
        for t in range(4):
            s = fin.tile([MT, 1], F32, name=f"smc{t}")
            nc.sync.dma_start(s[:], _r2(cc_out[CC_SM + MT * t:CC_SM + MT * (t + 1)], 1))
            smc.append(s)
        numr = fin.tile([1, N], F32)
        nc.sync.dma_start(numr[:], _r2(cc_out[CC_NUM:CC_NUM + N], N))
        smr = fin.tile([1, N], F32)
        nc.sync.dma_start(smr[:], _r2(cc_out[CC_SM:CC_SM + N], N))

        # scores row = cate * num / max(sm, 1)
        smx = fin.tile([1, N], F32)
        nc.vector.tensor_scalar(smx[:], smr[:], 1.0, None, op0=ALU.max)
        rcp = fin.tile([1, N], F32)
        nc.vector.reciprocal(rcp[:], smx[:])
        sc1 = fin.tile([1, N], F32)
        nc.vector.tensor_tensor(sc1[:], numr[:], rcp[:], op=ALU.mult)
        scores = fin.tile([1, N], F32)
        nc.vector.tensor_tensor(scores[:], sc1[:], cate_s[:], op=ALU.mult)

        scr_a = dram.tile([N], F32)   # rcomp bounce (column -> row)
        scr_b = dram.tile([N], F32)   # decay bounce
        dmt = []
        for t in range(4):
            u = work.tile([MT, N], F32, tag="u", name="u")
            # u = (sm[i] + sm[j]) - S[j,i]
            nc.vector.scalar_tensor_tensor(u[:], smb[:], smc[t][:], st[t][:],
                                           op0=ALU.add, op1=ALU.subtract)
            uc = work.tile([MT, N], F32, tag="uc", name="uc")
            nc.vector.tensor_scalar(uc[:], u[:], 1e-6, None, op0=ALU.max)
            rr = work.tile([MT, N], F32, tag="rr", name="rr")
            nc.vector.reciprocal(rr[:], uc[:])
            iou = work.tile([MT, N], F32, tag="iou", name="iou")
            nc.vector.tensor_tensor(iou[:], st[t][:], rr[:], op=ALU.mult)
            dt_ = work.tile([MT, N], F32, tag="dt", name="dt_")
            nc.vector.tensor_tensor(dt_[:], iou[:], maskt_s[t][:], op=ALU.mult)
            comp = fin.tile([MT, 1], F32, name=f"comp{t}")
            nc.vector.tensor_reduce(comp[:], dt_[:],
                                    axis=mybir.AxisListType.X, op=ALU.max)
            csq = fin.tile([MT, 1], F32, name=f"csq{t}")
            nc.vector.tensor_tensor(csq[:], comp[:], comp[:], op=ALU.mult)
            rcm = fin.tile([MT, 1], F32, name=f"rcm{t}")
            # 1/comp_matrix = exp(+SIGMA * comp^2)
            nc.scalar.activation(rcm[:], csq[:], AFT.Exp, scale=float(SIGMA))
            nc.sync.dma_start(_r2(scr_a[MT * t:MT * (t + 1)], 1), rcm[:])
            sq = work.tile([MT, N], F32, tag="sq", name="sq")
            nc.scalar.activation(sq[:], dt_[:], AFT.Square)
            dm = fin.tile([MT, N], F32, name=f"dm{t}")
            nc.scalar.activation(dm[:], sq[:], AFT.Exp, scale=float(-SIGMA))
            dmt.append(dm)

        rcb = fin.tile([MT, N], F32)
        nc.gpsimd.dma_start(rcb[:], bass.AP(tensor=scr_a.tensor,
                                            offset=scr_a.offset,
                                            ap=[[0, MT], [1, N]]))
        for t in range(4):
            ratio = work.tile([MT, N], F32, tag="ratio", name="ratio")
            nc.vector.tensor_tensor(ratio[:], dmt[t][:], rcb[:], op=ALU.mult)
            dec = fin.tile([MT, 1], F32, name=f"dec{t}")
            nc.vector.tensor_reduce(dec[:], ratio[:],
                                    axis=mybir.AxisListType.X, op=ALU.min)
            nc.sync.dma_start(_r2(scr_b[MT * t:MT * (t + 1)], 1), dec[:])
        decrow = fin.tile([1, N], F32)
        nc.sync.dma_start(decrow[:], _r2(scr_b[:], N))
        res = fin.tile([1, N], F32)
        nc.vector.tensor_tensor(res[:], scores[:], decrow[:], op=ALU.mult)
        nc.sync.dma_start(out_d[:], res[:])

    nc.compile()
    return nc


def _get_nc():
    if not _NC_CACHE:
        _NC_CACHE.append(_build_nc())
    return _NC_CACHE[0]


def _prep_inputs(cate_scores, seg_preds_x, seg_preds_y, cate_labels, x_inds,
                 y_inds):
    bf16 = ml_dtypes.bfloat16
    X = np.ascontiguousarray(np.asarray(seg_preds_x, np.float32).reshape(G, HW))
    Y = np.ascontiguousarray(np.asarray(seg_preds_y, np.float32).reshape(G, HW))
    xhi = X.astype(bf16)
    xlo = (X - xhi.astype(np.float32)).astype(bf16)
    yhi = Y.astype(bf16)
    ylo = (Y - yhi.astype(np.float32)).astype(bf16)

    xi = np.asarray(x_inds).astype(np.int64)
    yi = np.asarray(y_inds).astype(np.int64)
    lab = np.asarray(cate_labels).astype(np.int64)
    ohx = (np.arange(G)[:, None] == xi[None, :]).astype(bf16)
    ohy = (np.arange(G)[:, None] == yi[None, :]).astype(bf16)

    jj = np.arange(N)
    maskt = ((lab[None, :] == lab[:, None]) &
             (jj[None, :] < jj[:, None])).astype(np.float32).reshape(4, MT, N)
    cate = np.asarray(cate_scores, np.float32).reshape(1, N)

    def shard(a):
        out = np.zeros((G, PAD), a.dtype)
        return out

    in_maps = []
    for k in range(NCORES):
        sl = np.s_[:, k * PPC:(k + 1) * PPC]
        m = {}
        for name, arr in (("xhi", xhi), ("xlo", xlo), ("yhi", yhi),
                          ("ylo", ylo)):
            s = np.zeros((G, PAD), bf16)
            s[:, :PPC] = arr[sl]
            m[name] = s
        m["ohx"] = ohx
        m["ohy"] = ohy
        m["maskt"] = maskt
        m["cate"] = cate
        in_maps.append(m)
    return in_maps


def kernel(**inputs) -> np.ndarray:
    in_maps = _prep_inputs(**inputs)
    nc = _get_nc()
    res = run_bass_kernel_spmd(nc, in_maps, core_ids=list(range(NCORES)))
    return np.asarray(res.results[0]["out"], np.float32).reshape(N)


if __name__ == "__main__":
    # quick self-run with random inputs
    rng = np.random.default_rng(0)
    inputs = dict(
        cate_scores=rng.random(N, np.float32),
        seg_preds_x=rng.random((G, H, W), np.float32),
        seg_preds_y=rng.random((G, H, W), np.float32),
        cate_labels=rng.integers(0, 80, N),
        x_inds=rng.integers(0, G, N),
        y_inds=rng.integers(0, G, N),
    )
    out = kernel(**inputs)
    print(out[:10])
